# revision 19
# baseline (speedup 1.0000x reference)
"""DeepTEN encoding kernel for Trainium2 (8 NeuronCores, SPMD data-parallel over batch).

Math (per batch b):
    xf = x[b] viewed (D, N), N = H*W
    dist[n,k] = ||xf[:,n] - c[k]||^2 ;  logits = -scale * dist ;  A = softmax_k(logits)
    E[k,d] = sum_n A[n,k] * (xf[d,n] - c[k,d]) = (A^T X)[k,d] - colsum(A)[k]*c[k,d]

Device decomposition (softmax in (n-partitions, k-free) layout, 2048-n blocks):
    w = -scale (>0), maxs = max(w), wm = w - maxs
    The shifted logit  (wm_k*xsq_n + w_k*csq_k) - 2*w_k*<x_n,c_k>  is built
    entirely in PSUM by the PE:
      * a 64-row block-diagonal "seed" matmul supplies wm_k*xsq_n + w_k*csq_k
        for all 16 n-tiles of a block at once (4 contraction rows per tile:
        xsq_hi, xsq_lo, xsq_hi, ones against wm_hi, wm_hi, wm_lo, wcsq; the
        bf16 hi/lo splits keep the exponent error < ~3e-3)
      * 16 fp8(e4m3) x-tile-stationary matmuls accumulate -2*w_k*<x,c>;
        w1 = -2*w*c is pre-scaled by 64 into fp8 range and the exp rescales
        by 1/64 (ACT computes exp(psum/64) in one fused op)
    P = exp(psum/64) ;  S[n] = sum_k P ;  A = P / S
    psum_E[k, 0:D] += sum_n A[n,k]*xT[n,d]   (PE accumulates whole batch,
    psum_E[k, D]   += sum_n A[n,k]            ones-column fused colsum)

x is uploaded twice — fp8 (D,N) for the distance matmuls (quantization only
perturbs the softmax logits by ~0.6%) and bf16 pre-transposed tiles (p, gi, d)
for the aggregation matmuls — so no on-device transpose is needed; total HBM
read traffic ~= 0.75 bytes/elem of x. The mm2s of block j are emitted after
the softmax chain of block j+1 (software pipelining) and ping-pong between
two PSUM accumulators (summed on host) to avoid back-to-back accumulate
stalls; the finished accumulators are DMAed to DRAM straight out of PSUM.
"""
import os
import sys
import numpy as np

sys.path.insert(0, "/opt/trn_rl_repo")

import ml_dtypes  # noqa: E402

BF16 = ml_dtypes.bfloat16
FP8 = ml_dtypes.float8_e4m3

B, D, H, W = 32, 128, 128, 128
K = 32
N = H * W            # 16384
NCORES = 8
BPC = B // NCORES    # batches per core
TILN = 128           # n per tile (matmul stationary width)
NTIL = 16            # tiles per block
BLKN = TILN * NTIL   # 2048 n per block
NBLK = N // BLKN     # 8 blocks per batch
W1SCALE = 64.0       # fp8 range scale on w1, undone inside the exp

_CACHE = {}


def _build_module():
    from contextlib import ExitStack
    import concourse.tile as tile
    from concourse import bacc, mybir

    nc = bacc.Bacc("TRN2", target_bir_lowering=False, debug=False, num_devices=NCORES)
    bf = mybir.dt.bfloat16
    f32 = mybir.dt.float32
    f8 = mybir.dt.float8e4

    x_d = nc.dram_tensor("x", [BPC, D, N], f8, kind="ExternalInput").ap()
    # xt[b, p, gi, d] = x[b, d, gi*128 + p]
    xt_d = nc.dram_tensor("xt", [BPC, 128, N // TILN, D + 1], bf, kind="ExternalInput").ap()
    # xsq4[b, i*4+r, j, p]: per-block seed lhsT rows; r in {hi, lo, hi, ones}
    xsq4_d = nc.dram_tensor("xsq4", [BPC, 4 * NTIL, NBLK, 128], bf, kind="ExternalInput").ap()
    # bdg[i*4+r, i*K+k]: block-diagonal seed rhs = 64*[wm_hi, wm_hi, wm_lo, wcsq][k]
    bdg_d = nc.dram_tensor("bdg", [4 * NTIL, NTIL * K], bf, kind="ExternalInput").ap()
    w1_d = nc.dram_tensor("w1", [D, K], f8, kind="ExternalInput").ap()
    oute_d = nc.dram_tensor("out_e", [BPC, K, 2, D + 1], f32, kind="ExternalOutput").ap()

    with tile.TileContext(nc) as tc, ExitStack() as ctx:
        cpool = ctx.enter_context(tc.tile_pool(name="const", bufs=1))
        xpool = ctx.enter_context(tc.tile_pool(name="xblk", bufs=6))
        xtpool = ctx.enter_context(tc.tile_pool(name="xtblk", bufs=6))
        qpool = ctx.enter_context(tc.tile_pool(name="xsqb", bufs=2))
        ppool = ctx.enter_context(tc.tile_pool(name="pexp", bufs=5))
        npool = ctx.enter_context(tc.tile_pool(name="pnorm", bufs=5))
        vpool = ctx.enter_context(tc.tile_pool(name="small", bufs=4))
        ps_xc = ctx.enter_context(tc.tile_pool(name="ps_xc", bufs=3, space="PSUM"))
        ps_e = ctx.enter_context(tc.tile_pool(name="ps_e", bufs=2, space="PSUM"))

        w1_sb = cpool.tile([D, K], f8)
        nc.sync.dma_start(out=w1_sb[:], in_=w1_d[:, :])
        bdg_sb = cpool.tile([4 * NTIL, NTIL * K], bf)
        nc.sync.dma_start(out=bdg_sb[:], in_=bdg_d[:, :])

        # Software pipeline: mm2s of block j are emitted after the softmax
        # chain of block j+1, so the PE hides the chain latency.
        pending = []  # (b, pn_sb, xt2_sb, hb, i0, i1, finish)
        psum_es = {}
        xsq4_bs = {}
        first_mm2 = {}

        def emit_mm2s(b, pn_sb, xt2_sb, hb, i0, i1, finish):
            pe0, pe1 = psum_es[b]
            ff = first_mm2[b]
            for i in range(i0, i1):
                pp = i % 2
                nc.tensor.matmul(
                    (pe0, pe1)[pp][:],
                    lhsT=pn_sb[:, K * i : K * (i + 1)],
                    rhs=xt2_sb[:, hb + i, :],
                    start=ff[pp],
                    stop=(finish and i >= NTIL - 2),
                )
                ff[pp] = False
            if finish:
                e_sb = vpool.tile([K, 2, D + 1], f32, tag="e_out")
                nc.scalar.activation(
                    e_sb[:, 0, :], pe0[:], mybir.ActivationFunctionType.Copy
                )
                nc.vector.tensor_copy(e_sb[:, 1, :], pe1[:])
                nc.sync.dma_start(out=oute_d[b], in_=e_sb[:])

        def fetch_xsq4(b):
            xsq4_b = qpool.tile(
                [4 * NTIL, NBLK, 128], bf, name=f"xsq4_b{b}", tag="xsqb"
            )
            nc.sync.dma_start(out=xsq4_b[:], in_=xsq4_d[b])
            xsq4_bs[b] = xsq4_b

        fetch_xsq4(0)
        for gblk in range(BPC * NBLK):
            b, blk = divmod(gblk, NBLK)
            if blk == 0:
                if b + 1 < BPC:
                    fetch_xsq4(b + 1)  # prefetch next batch's seed rows
                psum_es[b] = (
                    ps_e.tile([K, D + 1], f32, tag="pe0", name=f"psum_e0_b{b}"),
                    ps_e.tile([K, D + 1], f32, tag="pe1", name=f"psum_e1_b{b}"),
                )
                first_mm2[b] = [True, True]
            if blk % 2 == 0:
                boff = blk * BLKN
                x2_sb = xpool.tile([D, 2 * BLKN], f8)
                nc.sync.dma_start(
                    out=x2_sb[:], in_=x_d[b][:, boff : boff + 2 * BLKN]
                )
                xt2_sb = xtpool.tile([128, 2 * NTIL, D + 1], bf)
                nc.scalar.dma_start(
                    out=xt2_sb[:],
                    in_=xt_d[b][:, blk * NTIL : (blk + 2) * NTIL, :],
                )
            hb = (blk % 2) * NTIL
            psum_xc = ps_xc.tile([128, NTIL * K], f32)
            nc.tensor.matmul(
                psum_xc[:],
                lhsT=xsq4_bs[b][:, blk, :],
                rhs=bdg_sb[:],
                start=True,
                stop=False,
                skip_group_check=True,
            )
            for i in range(NTIL):
                nc.tensor.matmul(
                    psum_xc[:, K * i : K * (i + 1)],
                    lhsT=x2_sb[:, (hb + i) * TILN : (hb + i + 1) * TILN],
                    rhs=w1_sb[:, :],
                    start=False,
                    stop=True,
                    skip_group_check=True,
                )

            # Softmax chain: exp (ACT) -> rowsum (Pool) -> 1/S, P*Sinv (DVE).
            # The very last block runs in two halves so the pipeline drain at
            # the end of the kernel pays half-size chain stages.
            last = gblk == BPC * NBLK - 1
            nh = 4 if last else 1
            ht = NTIL // nh
            p_sb = ppool.tile([128, NTIL * K], bf, tag="pexp")
            s_sb = vpool.tile([128, NTIL], f32, tag="s")
            sinv_sb = vpool.tile([128, NTIL], f32, tag="sinv")
            pn_sb = npool.tile([128, NTIL * K], bf, tag="pn")
            for h in range(nh):
                tsl = slice(h * ht, (h + 1) * ht)
                csl = slice(h * ht * K, (h + 1) * ht * K)
                nc.scalar.activation(
                    p_sb[:, csl],
                    psum_xc[:, csl],
                    mybir.ActivationFunctionType.Exp,
                    scale=1.0 / W1SCALE,
                )
                p3 = p_sb[:, csl].rearrange("p (i k) -> p i k", k=K)
                nc.vector.reduce_sum(s_sb[:, tsl], p3, axis=mybir.AxisListType.X)
                nc.vector.reciprocal_approx_fast(out=sinv_sb[:, tsl], in_=s_sb[:, tsl])
                nc.vector.tensor_tensor(
                    pn_sb[:, csl].rearrange("p (i k) -> p i k", k=K),
                    p3,
                    sinv_sb[:, tsl].broadcast_to([128, ht, K]),
                    op=mybir.AluOpType.mult,
                )
                pending.append(
                    (b, pn_sb, xt2_sb, hb, h * ht, (h + 1) * ht,
                     blk == NBLK - 1 and h == nh - 1)
                )
                if len(pending) > 1:
                    emit_mm2s(*pending.pop(0))

        while pending:
            emit_mm2s(*pending.pop(0))

    nc.compile()
    return nc


def _get_module():
    if "nc" not in _CACHE:
        _CACHE["nc"] = _build_module()
    return _CACHE["nc"]


def _host_prep(x, codewords, scale):
    x = np.asarray(x, dtype=np.float32)
    c = np.asarray(codewords, dtype=np.float32)
    s = np.asarray(scale, dtype=np.float32)

    w = -s                           # (K,) in (0, 1)
    maxs = float(w.max())
    w1 = (-2.0 * W1SCALE * (w[:, None] * c)).T.astype(FP8)  # (D, K) fp8, x64
    wm = w - maxs                                           # (K,) <= 0
    wm_hi = wm.astype(BF16).astype(np.float32)
    wm_lo = wm - wm_hi
    wcsq = w * (c * c).sum(axis=1)                          # (K,)

    xf = x.reshape(B, D, N)
    xsq = np.einsum("bdn,bdn->bn", xf, xf)                  # (B, N) fp32
    xsq_hi = xsq.astype(BF16)
    xsq_lo = (xsq - xsq_hi.astype(np.float32)).astype(BF16)
    # xsq4[b, i*4+r, j, p]: r = 0..3 -> (xsq_hi, xsq_lo, xsq_hi, 1); the seed
    # value at psum[p, (i,k)] is sum_r xsq4[b,i*4+r,j,p] * bdg[i*4+r, i*K+k]
    hi_r = np.ascontiguousarray(
        xsq_hi.reshape(B, NBLK, NTIL, 128).transpose(0, 2, 1, 3)
    )                                                       # (B, 16, NBLK, 128)
    lo_r = np.ascontiguousarray(
        xsq_lo.reshape(B, NBLK, NTIL, 128).transpose(0, 2, 1, 3)
    )
    xsq4 = np.empty((B, NTIL, 4, NBLK, 128), dtype=BF16)
    xsq4[:, :, 0] = hi_r
    xsq4[:, :, 1] = lo_r
    xsq4[:, :, 2] = hi_r
    xsq4[:, :, 3] = 1.0
    xsq4 = xsq4.reshape(B, 4 * NTIL, NBLK, 128)

    bdg = np.zeros((4 * NTIL, NTIL * K), dtype=BF16)
    rows = np.stack(
        [W1SCALE * wm_hi, W1SCALE * wm_lo, W1SCALE * wcsq]
    ).astype(BF16)                                          # (3, K)
    for i in range(NTIL):
        bdg[i * 4 + 0, i * K : (i + 1) * K] = rows[0]
        bdg[i * 4 + 1, i * K : (i + 1) * K] = rows[0]
        bdg[i * 4 + 2, i * K : (i + 1) * K] = rows[1]
        bdg[i * 4 + 3, i * K : (i + 1) * K] = rows[2]

    x8 = xf.astype(FP8)                                     # (B, D, N)
    # xt[b, p, gi, d] = xf[b, d, gi*128 + p];  xt[..., D] = 1.0 (fused colsum column)
    xt = np.ones((B, N // TILN, TILN, D + 1), dtype=BF16)
    xt[:, :, :, :D] = xf.transpose(0, 2, 1).reshape(B, N // TILN, TILN, D).astype(BF16)
    xt = np.ascontiguousarray(xt.transpose(0, 2, 1, 3))     # (B, 128, N/128, D+1)
    return x8, xt, xsq4, bdg, w1


def make_in_maps(x, codewords, scale):
    x8, xt, xsq4, bdg, w1 = _host_prep(x, codewords, scale)
    in_maps = []
    for ci in range(NCORES):
        sl = slice(BPC * ci, BPC * (ci + 1))
        in_maps.append(
            {
                "x": np.ascontiguousarray(x8[sl]),
                "xt": np.ascontiguousarray(xt[sl]),
                "xsq4": np.ascontiguousarray(xsq4[sl]),
                "bdg": bdg,
                "w1": w1,
            }
        )
    return in_maps


def finish_output(results, codewords):
    c = np.asarray(codewords, dtype=np.float32)
    out = np.zeros((B, K * D), dtype=np.float32)
    for ci, r in enumerate(results):
        for bb in range(BPC):
            e_parts = r["out_e"][bb][:, 0, :] + r["out_e"][bb][:, 1, :]  # (K, D+1)
            e = e_parts[:, :D] - e_parts[:, D : D + 1] * c
            out[BPC * ci + bb] = e.reshape(-1)
    return out


def kernel(x, codewords, scale):
    from concourse.bass_utils import run_bass_kernel_spmd
    from concourse.bass_interp import get_hw_module

    nc = _get_module()
    in_maps = make_in_maps(x, codewords, scale)

    old_m = nc.m
    nc.m = get_hw_module(nc.m)
    try:
        res = run_bass_kernel_spmd(nc, in_maps, core_ids=list(range(NCORES)))
    finally:
        nc.m = old_m
    return finish_output(res.results, codewords)
